# revision 3
# baseline (speedup 1.0000x reference)
"""Trainium2 Bass kernel: batched single-channel 3x3 valid conv, 16 output channels.

reference: x [32, 512, 512] f32, kernels [16, 3, 3] f32
           -> out [32, 16, 510, 510] f32  (cross-correlation, VALID, stride 1)

Strategy (memory-regime problem: output is 532 MB/8 cores = 66.6 MB/core):
  - Data-parallel: 4 images per core across 8 cores; kernels replicated.
  - 32-row output blocks (15 full blocks + one final block at row 478 that
    overlaps the previous by 2 rows and rewrites identical values). Per block
    one PE matmul per channel-group: contraction K = 3 column-shifts x 34
    input rows = 102 against a host-precomputed banded lhsT [102, 128]
    (M = 4 channels x 32 rows -> full 128 PSUM partitions). rhs [102, 510]
    loaded by ONE input DMA (gpsimd/SWDGE ring, so input loads never queue
    behind output stores) whose source AP reads the 3 overlapping column
    windows. Matmuls run in float32r: 1 PE cycle/row at N>=256 vs 4 for
    plain f32 (~59us vs ~218us PE time per core).
  - PSUM [128, 510] tiles are copied (ScalarE/VectorE alternating) into
    per-group staging tiles [128, 8*510]; each half-image flushes with
    [128-partition, ~2 MB] sync-ring dma_starts. 128-partition calls engage
    all 16 SDMA ports (the old 30-partition flushes ran at ~1/4 bandwidth,
    which was the baseline bottleneck).
"""

import numpy as np

import concourse.bass as bass
import concourse.mybir as mybir
import concourse.tile as tile
from concourse import bacc
from concourse.bass_utils import run_bass_kernel_spmd

N_CORES = 8
B, H, W = 32, 512, 512
KN, KS = 16, 3
OH, OW = H - KS + 1, W - KS + 1  # 510, 510
B_LOC = B // N_CORES  # 4

ROWS = 32                # output rows per block
IN_ROWS = ROWS + KS - 1  # 34 input rows per block
KDIM = KS * IN_ROWS      # 102 contraction
NBLK = 16                # 15 full blocks + overlapping tail block
KG = 4                   # channels per matmul group
N_GROUPS = KN // KG      # 4
M = KG * ROWS            # 128 psum partitions
BLK_STARTS = [j * ROWS for j in range(NBLK - 1)] + [OH - ROWS]  # last = 478

F32 = mybir.dt.float32


def _build_nc(use_f32r=True, in_ring="gpsimd", out_ring="sync"):
    in_dt = mybir.dt.float32r if use_f32r else F32
    nc = bacc.Bacc("TRN2", target_bir_lowering=False, debug=False)
    x_t = nc.dram_tensor("x", [B_LOC, H, W], in_dt, kind="ExternalInput")
    w_t = nc.dram_tensor("w", [KDIM, N_GROUPS * M], in_dt, kind="ExternalInput")
    out_t = nc.dram_tensor("out", [B_LOC, KN, OH, OW], F32, kind="ExternalOutput")

    CH_STRIDE = OH * OW  # dram elems between channels of one image

    with tile.TileContext(nc) as tc:
        with (
            tc.tile_pool(name="wpool", bufs=1) as wpool,
            tc.tile_pool(name="inpool", bufs=4) as inpool,
            tc.tile_pool(name="psum", bufs=8, space="PSUM") as psum_pool,
            tc.tile_pool(name="stage", bufs=6) as stage_pool,
        ):
            wt = wpool.tile([KDIM, N_GROUPS * M], in_dt)
            nc.sync.dma_start(out=wt[:, :], in_=w_t[:, :])
            cp = 0
            for b in range(B_LOC):
                for j in range(NBLK):
                    r = BLK_STARTS[j]
                    base = inpool.tile([KDIM, OW], in_dt)
                    src = x_t.ap()[b]  # [H, W]
                    getattr(nc, in_ring).dma_start(
                        out=base[:, :],
                        in_=bass.AP(
                            src.tensor,
                            src.offset + r * W,
                            [[1, KS], [W, IN_ROWS], [1, OW]],
                        ),
                    )
                    dst_root = out_t.ap()[b]
                    for g in range(N_GROUPS):
                        ps = psum_pool.tile([M, OW], F32)
                        nc.tensor.matmul(
                            ps[:, :],
                            lhsT=wt[:, g * M : (g + 1) * M],
                            rhs=base[:, :],
                            start=True,
                            stop=True,
                        )
                        st = stage_pool.tile([M, OW], F32, tag=f"st{g}")
                        if cp % 2 == 0:
                            nc.scalar.copy(out=st[:, :], in_=ps[:, :])
                        else:
                            nc.vector.tensor_copy(out=st[:, :], in_=ps[:, :])
                        cp += 1
                        # flush [128 partitions, 510] = 261 KB: DRAM dims
                        # (k:4, y:32, x:510) <= 3-dim DMA AP limit; 128
                        # partitions engage all 16 SDMA ports, back-to-back
                        # calls pipeline the fixed completion latency.
                        getattr(nc, out_ring).dma_start(
                            out=bass.AP(
                                dst_root.tensor,
                                dst_root.offset + g * KG * CH_STRIDE + r * OW,
                                [[CH_STRIDE, KG], [OW, ROWS], [1, OW]],
                            ),
                            in_=st[:, :],
                        )
    nc.finalize()
    return nc


def _pack_weights(kernels: np.ndarray) -> np.ndarray:
    """lhsT pack: w[dx*IN_ROWS + y + dy, g*M + k*ROWS + y] = kernels[g*KG+k, dy, dx].

    psum[k*ROWS + y, n] = sum_{dx, y'} lhsT[dx*IN_ROWS + y', k*ROWS + y]
                                       * x[r + y', n + dx]
                        = sum_{dy, dx} kernels[g*KG+k, dy, dx] * x[r + y + dy, n + dx]
    """
    w = np.zeros((KDIM, N_GROUPS * M), np.float32)
    y = np.arange(ROWS)
    for g in range(N_GROUPS):
        for dx in range(KS):
            for k in range(KG):
                for dy in range(KS):
                    w[dx * IN_ROWS + y + dy, g * M + k * ROWS + y] = kernels[
                        g * KG + k, dy, dx
                    ]
    return w


def run(x, kernels, trace=False, **build_kwargs):
    x = np.ascontiguousarray(np.asarray(x, dtype=np.float32))
    kernels = np.asarray(kernels, dtype=np.float32)
    assert x.shape == (B, H, W) and kernels.shape == (KN, KS, KS)
    nc = _build_nc(**build_kwargs)
    wp = _pack_weights(kernels)
    in_maps = [
        {"x": x[c * B_LOC : (c + 1) * B_LOC], "w": wp} for c in range(N_CORES)
    ]
    res = run_bass_kernel_spmd(
        nc, in_maps, core_ids=list(range(N_CORES)), trace=trace
    )
    out = np.concatenate([res.results[c]["out"] for c in range(N_CORES)], axis=0)
    return out, res


def kernel(x, kernels):
    out, _ = run(x, kernels)
    return out


# revision 6
# speedup vs baseline: 1.8483x; 1.8483x over previous
"""Trainium2 Bass kernel: batched single-channel 3x3 valid conv, 16 output channels.

reference: x [32, 512, 512] f32, kernels [16, 3, 3] f32
           -> out [32, 16, 510, 510] f32  (cross-correlation, VALID, stride 1)

Strategy (memory-regime: output is 532 MB / 8 cores = 66.6 MB/core; HW
microbenchmarks showed per-dma_start ring stall of ~2-3 us regardless of
size, so few & huge output DMAs win):
  - Data-parallel: 4 images per core across 8 cores; kernels replicated.
  - Band-major tiling: 126-row output bands (4 bands + 6-row tail per
    image). Per (band, channel): PSUM [126, 510] accumulated by 3 matmuls,
    one per kernel column-shift dx, lhsT [128, 126] banded in dy,
    rhs = base[:, dx:dx+510] column-slices of ONE raw input tile
    [128, 512] (input loaded once, no 3x im2col replication).
  - float32r matmuls: 1 PE cycle/row at N>=510 (plain fp32 is 4).
  - PSUM -> SBUF band staging [126, 16*510] (ScalarE/VectorE alternating);
    one 4.1 MB output DMA per band: partition = 126 consecutive rows,
    free = (channel:16, x:510) -- 3-dim AP, ~line-rate HBM writes,
    only 16 big + 4 tail output DMAs per core.
  - 6-row tail per image: one composite matmul (M = 16ch x 6rows = 96,
    K = 3dx x 8rows = 24 im2col-style) + one [96, 510] flush.
"""

import numpy as np

import concourse.bass as bass
import concourse.mybir as mybir
import concourse.tile as tile
from concourse import bacc
from concourse.bass_utils import run_bass_kernel_spmd

N_CORES = 8
B, H, W = 32, 512, 512
KN, KS = 16, 3
OH, OW = H - KS + 1, W - KS + 1  # 510, 510
B_LOC = B // N_CORES  # 4

ROWS = 126               # output rows per band
IN_ROWS = 128            # input rows loaded per band
NBAND = 4                # bands cover rows 0..503
TAIL = OH - NBAND * ROWS  # 6 tail rows (504..509)
T_IN = TAIL + KS - 1     # 8 input rows for tail
T_K = KS * T_IN          # 24 tail contraction
T_M = KN * TAIL          # 96 tail psum partitions

F32 = mybir.dt.float32


def _build_nc(use_f32r=True, in_ring="gpsimd", out_ring="sync"):
    in_dt = mybir.dt.float32r if use_f32r else F32
    nc = bacc.Bacc("TRN2", target_bir_lowering=False, debug=False)
    x_t = nc.dram_tensor("x", [B_LOC, H, W], in_dt, kind="ExternalInput")
    # band weights: per (ch, dx) a [128, 126] banded lhsT, packed along cols
    w_t = nc.dram_tensor("w", [IN_ROWS, KN * KS * ROWS], in_dt, kind="ExternalInput")
    w2_t = nc.dram_tensor("w2", [T_K, T_M], in_dt, kind="ExternalInput")
    out_t = nc.dram_tensor("out", [B_LOC, KN, OH, OW], F32, kind="ExternalOutput")

    CH_STRIDE = OH * OW  # dram elems between channels of one image

    with tile.TileContext(nc) as tc:
        with (
            tc.tile_pool(name="wpool", bufs=1) as wpool,
            tc.tile_pool(name="inpool", bufs=3) as inpool,
            tc.tile_pool(name="psum", bufs=6, space="PSUM") as psum_pool,
            tc.tile_pool(name="psumt", bufs=2, space="PSUM") as psumt_pool,
            tc.tile_pool(name="stage", bufs=2) as stage_pool,
        ):
            wt = wpool.tile([IN_ROWS, KN * KS * ROWS], in_dt)
            nc.sync.dma_start(out=wt[:, :], in_=w_t[:, :])
            wt2 = wpool.tile([T_K, T_M], in_dt)
            nc.sync.dma_start(out=wt2[:, :], in_=w2_t[:, :])
            cp = 0
            for b in range(B_LOC):
                src = x_t.ap()[b]  # [H, W]
                dst_root = out_t.ap()[b]
                for band in range(NBAND):
                    r = band * ROWS
                    base = inpool.tile([IN_ROWS, W], in_dt)
                    getattr(nc, in_ring).dma_start(
                        out=base[:, :],
                        in_=bass.AP(
                            src.tensor, src.offset + r * W, [[W, IN_ROWS], [1, W]]
                        ),
                    )
                    st = stage_pool.tile([ROWS, KN * OW], F32, tag="band")
                    for ch in range(KN):
                        ps = psum_pool.tile([ROWS, OW], F32)
                        for dx in range(KS):
                            c0 = (ch * KS + dx) * ROWS
                            nc.tensor.matmul(
                                ps[:, :],
                                lhsT=wt[:, c0 : c0 + ROWS],
                                rhs=base[:, dx : dx + OW],
                                start=(dx == 0),
                                stop=(dx == KS - 1),
                            )
                        dst = st[:, ch * OW : (ch + 1) * OW]
                        if cp % 2 == 0:
                            nc.scalar.copy(out=dst, in_=ps[:, :])
                        else:
                            nc.vector.tensor_copy(out=dst, in_=ps[:, :])
                        cp += 1
                    # one 4.1 MB flush: partition = 126 consecutive rows,
                    # free = (channel, x)
                    getattr(nc, out_ring).dma_start(
                        out=bass.AP(
                            dst_root.tensor,
                            dst_root.offset + r * OW,
                            [[OW, ROWS], [CH_STRIDE, KN], [1, OW]],
                        ),
                        in_=st[:, :],
                    )
                # 6-row tail, all channels in one matmul
                tbase = inpool.tile([T_K, OW], in_dt, tag="tail")
                getattr(nc, in_ring).dma_start(
                    out=tbase[:, :],
                    in_=bass.AP(
                        src.tensor,
                        src.offset + NBAND * ROWS * W,
                        [[1, KS], [W, T_IN], [1, OW]],
                    ),
                )
                tps = psumt_pool.tile([T_M, OW], F32, tag="tailps")
                nc.tensor.matmul(
                    tps[:, :], lhsT=wt2[:, :], rhs=tbase[:, :], start=True, stop=True
                )
                tst = stage_pool.tile([T_M, OW], F32, tag="tailst")
                if cp % 2 == 0:
                    nc.scalar.copy(out=tst[:, :], in_=tps[:, :])
                else:
                    nc.vector.tensor_copy(out=tst[:, :], in_=tps[:, :])
                cp += 1
                getattr(nc, out_ring).dma_start(
                    out=bass.AP(
                        dst_root.tensor,
                        dst_root.offset + NBAND * ROWS * OW,
                        [[CH_STRIDE, KN], [OW, TAIL], [1, OW]],
                    ),
                    in_=tst[:, :],
                )
    nc.finalize()
    return nc


def _pack_weights(kernels: np.ndarray):
    """Band lhsT pack: w[y', (ch*KS + dx)*ROWS + y] = kernels[ch, y'-y, dx]
    for 0 <= y'-y <= 2 (psum[y, n] accumulates over dx of
    sum_{y'} lhsT[y', y] * x[r+y', n+dx]).

    Tail pack: w2[dx*T_IN + y + dy, ch*TAIL + y] = kernels[ch, dy, dx].
    """
    w = np.zeros((IN_ROWS, KN * KS * ROWS), np.float32)
    y = np.arange(ROWS)
    for ch in range(KN):
        for dx in range(KS):
            for dy in range(KS):
                w[y + dy, (ch * KS + dx) * ROWS + y] = kernels[ch, dy, dx]
    w2 = np.zeros((T_K, T_M), np.float32)
    yt = np.arange(TAIL)
    for ch in range(KN):
        for dx in range(KS):
            for dy in range(KS):
                w2[dx * T_IN + yt + dy, ch * TAIL + yt] = kernels[ch, dy, dx]
    return w, w2


def make_in_maps(x, kernels):
    wp, wp2 = _pack_weights(kernels)
    return [
        {"x": x[c * B_LOC : (c + 1) * B_LOC], "w": wp, "w2": wp2}
        for c in range(N_CORES)
    ]


def run(x, kernels, trace=False, **build_kwargs):
    x = np.ascontiguousarray(np.asarray(x, dtype=np.float32))
    kernels = np.asarray(kernels, dtype=np.float32)
    assert x.shape == (B, H, W) and kernels.shape == (KN, KS, KS)
    nc = _build_nc(**build_kwargs)
    in_maps = make_in_maps(x, kernels)
    res = run_bass_kernel_spmd(
        nc, in_maps, core_ids=list(range(N_CORES)), trace=trace
    )
    out = np.concatenate([res.results[c]["out"] for c in range(N_CORES)], axis=0)
    return out, res


def kernel(x, kernels):
    out, _ = run(x, kernels)
    return out
